# revision 18
# baseline (speedup 1.0000x reference)
"""MHA kernel for Trainium2, 8 NeuronCores.

Problem (hardcoded): B=4, LQ=LK=2048, D=1024, 16 heads x head_dim 64, f32,
mask all-ones (no masking).

Sharding: core c = 2*b + g handles batch b, head-group g (8 heads).
Each core computes q/k/v projections for its 512 head-columns, attention for
its 8 heads (as 4 row-tiled pairs), and a partial out-projection against its
512 rows of Wo. Host sums the two partials per batch.

All matmuls in bf16 (f32 PSUM accumulation). Exp on ScalarE is the
critical resource (~271us/core); emission interleaves projections with
attention kt-chunks so the exp stream starts as early as possible.

Layouts on chip:
  qhT/khT: (d=128 per head-pair on partitions, seq free)   [projection output]
  vext:    (seq on partitions, per-head 64 v-cols + ones col) -> softmax
           denominator comes free as row 64 of the P@V PSUM (M=65 stationary)
  S_T:     (k on partitions, q free); exp on ScalarE straight out of PSUM,
           head pairs computed concurrently via tile_position row-tiling
  O_T:     (head-dim on partitions, q free) feeds out-proj as lhsT directly
"""
import sys

import numpy as np

try:
    import concourse  # noqa: F401
except ImportError:
    sys.path.insert(0, "/opt/trn_rl_repo")

B, L, D = 4, 2048, 1024
NH, HD = 16, 64
GH = NH // 2          # heads per core group = 8
GW = GH * HD          # head-group width = 512
NPAIR = GH // 2       # head pairs per core = 4
KT = D // 128         # contraction tiles for projections = 8
ST = L // 128         # seq tiles of 128 = 16
NQ = L // 512         # seq tiles of 512 = 4

_state = {}


def _build_nc():
    from concourse import bacc, tile, mybir

    f32 = mybir.dt.float32
    bf16 = mybir.dt.bfloat16
    Exp = mybir.ActivationFunctionType.Exp

    nc = bacc.Bacc(None, target_bir_lowering=False, debug=False)
    xqT_d = nc.dram_tensor("xqT", [D, L], bf16, kind="ExternalInput")
    xkT_d = nc.dram_tensor("xkT", [D, L], bf16, kind="ExternalInput")
    xvT_d = nc.dram_tensor("xvT", [D, L], bf16, kind="ExternalInput")
    wq_d = nc.dram_tensor("wq", [D, GW], bf16, kind="ExternalInput")
    wk_d = nc.dram_tensor("wk", [D, GW], bf16, kind="ExternalInput")
    wv_d = nc.dram_tensor("wv", [D, GW], bf16, kind="ExternalInput")
    wo_d = nc.dram_tensor("wo", [GW, D], bf16, kind="ExternalInput")
    out_d = nc.dram_tensor("out", [L, D], f32, kind="ExternalOutput")

    with tile.TileContext(nc) as tc:
        with (
            tc.tile_pool(name="persist", bufs=1) as pers,
            tc.tile_pool(name="w", bufs=1) as wpool,
            tc.tile_pool(name="x", bufs=5) as xpool,
            tc.tile_pool(name="pt", bufs=12) as ptpool,
            tc.tile_pool(name="ev", bufs=2) as evpool,
            tc.tile_pool(name="small", bufs=3) as smpool,
            tc.tile_pool(name="ps_st", bufs=2, space="PSUM") as stpool,
            tc.tile_pool(name="ps_o", bufs=1, space="PSUM") as popool,
            tc.tile_pool(name="ps_pj", bufs=2, space="PSUM") as pjpool,
        ):
            # ---- persistent tiles -------------------------------------
            qhT = [pers.tile([128, L], bf16, tag=f"qhT{p}", name=f"qhT{p}") for p in range(NPAIR)]
            khT = [pers.tile([128, L], bf16, tag=f"khT{p}", name=f"khT{p}") for p in range(NPAIR)]
            vext = pers.tile([128, ST, GH, 65], bf16, tag="vext")
            oT = [[pers.tile([128, 512], bf16, tag=f"oT{p}_{n}", name=f"oT{p}_{n}")
                   for n in range(NQ)] for p in range(NPAIR)]

            # weights (resident; gpsimd queue so x DMAs own the sync queue)
            wk = wpool.tile([128, KT, GW], bf16, tag="wk")
            nc.gpsimd.dma_start(wk[:], wk_d.rearrange("(t p) m -> p t m", p=128))
            wv = wpool.tile([128, KT, GW], bf16, tag="wv")
            nc.gpsimd.dma_start(wv[:], wv_d.rearrange("(t p) m -> p t m", p=128))
            wq = wpool.tile([128, KT, GW], bf16, tag="wq")
            nc.gpsimd.dma_start(wq[:], wq_d.rearrange("(t p) m -> p t m", p=128))
            wo = wpool.tile([128, NPAIR, D], bf16, tag="wo")
            nc.gpsimd.dma_start(wo[:], wo_d.rearrange("(t p) m -> p t m", p=128))

            # ones columns of vext (rowsum trick); memset can't write bf16
            ones_f = smpool.tile([128, ST * GH], f32, tag="ones")
            nc.any.memset(ones_f[:], 1.0)
            nc.vector.tensor_copy(
                vext[:, :, :, 64],
                ones_f[:].rearrange("p (a b) -> p a b", a=ST))

            # ---- emission helpers -------------------------------------
            def load_x(x_dram, n, nm):
                xt = xpool.tile([128, KT, 512], bf16, tag="x", name=f"x{nm}{n}")
                nc.sync.dma_start(
                    xt[:], x_dram.rearrange("(t p) m -> p t m", p=128)[
                        :, :, n * 512:(n + 1) * 512])
                return xt

            def qk_chain(xt, w, dst, p, n):
                # one (pair, 512-wide seq chunk) of a q/k projection
                ps = pjpool.tile([128, 512], f32, tag="pj", name=f"pj{p}_{n}")
                for kt in range(KT):
                    nc.tensor.matmul(
                        ps[:], w[:, kt, p * 128:(p + 1) * 128], xt[:, kt, :],
                        start=(kt == 0), stop=(kt == KT - 1))
                nc.vector.tensor_copy(dst[p][:, n * 512:(n + 1) * 512], ps[:])

            def v_chunk(xt, mb):
                # 4 seq-tiles of the v projection -> vext rows
                for mi in range(4):
                    m = mb * 4 + mi
                    ps = pjpool.tile([128, 512], f32, tag="pj", name=f"pv{m}")
                    for kt in range(KT):
                        nc.tensor.matmul(
                            ps[:], xt[:, kt, mi * 128:(mi + 1) * 128],
                            wv[:, kt, :],
                            start=(kt == 0), stop=(kt == KT - 1))
                    nc.vector.tensor_copy(
                        vext[:, m, :, 0:64],
                        ps[:].rearrange("p (h d) -> p h d", h=GH))

            DEMOTE = -200

            def att_block(p, n):
                psO = popool.tile([128, 2, 512], f32, tag="psO", name=f"psO{p}_{n}")
                pts = {}

                def pv_batch(kts):
                    # P@V accumulation in same-config runs: keeps the PE in
                    # one array configuration (full K=128) so the row-tiled
                    # S_T pairs elsewhere stay concurrent and PVs stay 1 cyc/row
                    for g in range(2):
                        for kt in kts:
                            nc.tensor.matmul(
                                psO[0:65, g, :],
                                vext[:, kt, 2 * p + g, :],
                                pts[kt][:, g, :],
                                start=(kt == 0), stop=(kt == ST - 1))
                    for kt in kts:
                        del pts[kt]

                for kt in range(ST):
                    ps = stpool.tile([128, 2, 512], f32, tag="st", name=f"st{p}_{n}_{kt}")
                    for g, base in ((0, 0), (1, 64)):
                        nc.tensor.matmul(
                            ps[:, g, :],
                            khT[p][base:base + 64, kt * 128:(kt + 1) * 128],
                            qhT[p][base:base + 64, n * 512:(n + 1) * 512],
                            start=True, stop=True,
                            tile_position=(base, 0))
                    pt = ptpool.tile([128, 2, 512], bf16, tag="pt", name=f"pt{p}_{n}_{kt}")
                    nc.scalar.activation(pt[:], ps[:], Exp, scale=0.125)
                    pts[kt] = pt
                    if kt == 7:
                        pv_batch(range(0, 4))
                    elif kt == 11:
                        pv_batch(range(4, 8))
                pv_batch(range(8, ST))
                # evict psO to SBUF fast (frees the PSUM slot for the next
                # block) and normalize from the SBUF copy off-path
                sbo = smpool.tile([65, 2, 512], f32, tag="sbo")
                nc.vector.tensor_copy(sbo[:], psO[0:65, :, :])
                with tc.high_priority(DEMOTE):
                    for g in range(2):
                        rec = smpool.tile([1, 512], f32, tag="rec")
                        nc.vector.reciprocal(rec[:], sbo[64:65, g, :])
                        bc = smpool.tile([64, 512], f32, tag="bc")
                        nc.gpsimd.partition_broadcast(bc[:], rec[:])
                        nc.vector.tensor_mul(
                            oT[p][n][g * 64:(g + 1) * 64, :],
                            sbo[0:64, g, :], bc[:])

            def out_tile(mq, nn):
                ps = pjpool.tile([128, 512], f32, tag="pj", name=f"po{mq}_{nn}")
                for p in range(NPAIR):
                    nc.tensor.matmul(
                        ps[:],
                        oT[p][mq // 4][:, (mq % 4) * 128:(mq % 4 + 1) * 128],
                        wo[:, p, nn * 512:(nn + 1) * 512],
                        start=(p == 0), stop=(p == NPAIR - 1))
                ev = evpool.tile([128, 512], f32, tag="ev")
                nc.vector.tensor_copy(ev[:], ps[:])
                nc.sync.dma_start(
                    out_d[mq * 128:(mq + 1) * 128,
                          nn * 512:(nn + 1) * 512], ev[:])

            def k_chain(p, n):
                xt = load_x(xkT_d, n, f"k{p}_")
                qk_chain(xt, wk, khT, p, n)

            # ---- head: the minimum to start block (0,0) ---------------
            k_chain(0, 0)
            xq0 = load_x(xqT_d, 0, "q")
            qk_chain(xq0, wq, qhT, 0, 0)
            xv = load_x(xvT_d, 0, "v")
            v_chunk(xv, 0)
            xv = load_x(xvT_d, 1, "v")
            v_chunk(xv, 1)
            # rest of pair-0 k and v as demoted fillers (dep-ordered)
            with tc.high_priority(DEMOTE):
                for n in range(1, NQ):
                    k_chain(0, n)
                for mb in range(2, 4):
                    xv = load_x(xvT_d, mb, "v")
                    v_chunk(xv, mb)

            # ---- stripe 0, with next pair's projections as fillers ----
            for p in range(NPAIR):
                att_block(p, 0)
                if p < NPAIR - 1:
                    with tc.high_priority(DEMOTE):
                        for n in range(NQ):
                            k_chain(p + 1, n)
                        qk_chain(xq0, wq, qhT, p + 1, 0)

            # ---- stripes 1..3 -----------------------------------------
            for n in range(1, NQ):
                with tc.high_priority(DEMOTE):
                    xq = load_x(xqT_d, n, f"q{n}")
                    for p in range(NPAIR):
                        qk_chain(xq, wq, qhT, p, n)
                for p in range(NPAIR):
                    att_block(p, n)
                    with tc.high_priority(DEMOTE):
                        out_tile((n - 1) * 4 + p, 0)
                        out_tile((n - 1) * 4 + p, 1)
            for mq in range(12, 16):
                out_tile(mq, 0)
                out_tile(mq, 1)

    nc.compile()
    return nc


def _get_runner():
    """Build the bass module once and wrap it in a cached jitted executable
    (mirrors bass2jax.run_bass_via_pjrt, but reusable across calls)."""
    if "runner" in _state:
        return _state["runner"]

    import jax
    from jax.sharding import Mesh, PartitionSpec
    from jax.experimental.shard_map import shard_map
    from concourse import bass2jax, mybir

    nc = _build_nc()
    bass2jax.install_neuronx_cc_hook()

    partition_name = nc.partition_id_tensor.name if nc.partition_id_tensor else None
    in_names, out_names, out_avals, zero_shapes = [], [], [], []
    for alloc in nc.m.functions[0].allocations:
        if not isinstance(alloc, mybir.MemoryLocationSet):
            continue
        name = alloc.memorylocations[0].name
        if alloc.kind == "ExternalInput":
            if name == partition_name:
                continue
            in_names.append(name)
        elif alloc.kind == "ExternalOutput":
            shape = tuple(alloc.tensor_shape)
            dtype = mybir.dt.np(alloc.dtype)
            out_names.append(name)
            out_avals.append(jax.core.ShapedArray(shape, dtype))
            zero_shapes.append((shape, dtype))
    n_params = len(in_names)
    all_in_names = in_names + out_names
    if partition_name is not None:
        all_in_names = all_in_names + [partition_name]

    def _body(*args):
        operands = list(args)
        if partition_name is not None:
            operands.append(bass2jax.partition_id_tensor())
        outs = bass2jax._bass_exec_p.bind(
            *operands,
            out_avals=tuple(out_avals),
            in_names=tuple(all_in_names),
            out_names=tuple(out_names),
            lowering_input_output_aliases=(),
            sim_require_finite=True,
            sim_require_nnan=True,
            nc=nc,
        )
        return tuple(outs)

    devices = jax.devices()[:8]
    mesh = Mesh(np.asarray(devices), ("core",))
    nio = n_params + len(out_names)
    fn = jax.jit(
        shard_map(_body, mesh=mesh,
                  in_specs=(PartitionSpec("core"),) * nio,
                  out_specs=(PartitionSpec("core"),) * len(out_names),
                  check_rep=False),
        donate_argnums=tuple(range(n_params, nio)),
        keep_unused=True,
    )

    def run(in_maps):
        concat_in = [
            np.concatenate([np.asarray(in_maps[c][name]) for c in range(8)], axis=0)
            for name in in_names
        ]
        zeros = [np.zeros((8 * s[0],) + tuple(s[1:]), dt) for s, dt in zero_shapes]
        outs = fn(*concat_in, *zeros)
        result = []
        for c in range(8):
            m = {}
            for i, name in enumerate(out_names):
                rows = zero_shapes[i][0][0]
                m[name] = np.asarray(outs[i][c * rows:(c + 1) * rows])
            result.append(m)
        return result

    _state["runner"] = run
    return run


def _make_in_maps(q, k, v, Wq, Wk, Wv, Wo):
    from concourse import mybir

    bf16 = mybir.dt.np(mybir.dt.bfloat16)
    in_maps = []
    for c in range(8):
        b, g = c // 2, c % 2
        cols = slice(g * GW, (g + 1) * GW)
        in_maps.append({
            "xqT": np.asarray(q[b].T, bf16),
            "xkT": np.asarray(k[b].T, bf16),
            "xvT": np.asarray(v[b].T, bf16),
            "wq": np.asarray(Wq[:, cols], bf16),
            "wk": np.asarray(Wk[:, cols], bf16),
            "wv": np.asarray(Wv[:, cols], bf16),
            "wo": np.asarray(Wo[cols, :], bf16),
        })
    return in_maps


def kernel(q, k, v, mask, Wq, Wk, Wv, Wo):
    q = np.asarray(q, np.float32)
    k = np.asarray(k, np.float32)
    v = np.asarray(v, np.float32)
    Wq = np.asarray(Wq, np.float32)
    Wk = np.asarray(Wk, np.float32)
    Wv = np.asarray(Wv, np.float32)
    Wo = np.asarray(Wo, np.float32)
    # mask is all-ones for this problem (spec fill=ones); no masking applied.

    run = _get_runner()
    res = run(_make_in_maps(q, k, v, Wq, Wk, Wv, Wo))
    out = np.empty((B, L, D), np.float32)
    for b in range(B):
        out[b] = res[2 * b]["out"] + res[2 * b + 1]["out"]
    return out


# revision 19
# speedup vs baseline: 1.1071x; 1.1071x over previous
"""MHA kernel for Trainium2, 8 NeuronCores.

Problem (hardcoded): B=4, LQ=LK=2048, D=1024, 16 heads x head_dim 64, f32,
mask all-ones (no masking).

Sharding: core c = 2*b + g handles batch b, head-group g (8 heads).
Each core computes q/k/v projections for its 512 head-columns, attention for
its 8 heads (as 4 row-tiled pairs), and a partial out-projection against its
512 rows of Wo. Host sums the two partials per batch.

All matmuls in bf16 (f32 PSUM accumulation). Exp on ScalarE is the
critical resource (~271us/core); emission interleaves projections with
attention kt-chunks so the exp stream starts as early as possible.

Layouts on chip:
  qhT/khT: (d=128 per head-pair on partitions, seq free)   [projection output]
  vext:    (seq on partitions, per-head 64 v-cols + ones col) -> softmax
           denominator comes free as row 64 of the P@V PSUM (M=65 stationary)
  S_T:     (k on partitions, q free); exp on ScalarE straight out of PSUM,
           head pairs computed concurrently via tile_position row-tiling
  O_T:     (head-dim on partitions, q free) feeds out-proj as lhsT directly
"""
import sys

import numpy as np

try:
    import concourse  # noqa: F401
except ImportError:
    sys.path.insert(0, "/opt/trn_rl_repo")

B, L, D = 4, 2048, 1024
NH, HD = 16, 64
GH = NH // 2          # heads per core group = 8
GW = GH * HD          # head-group width = 512
NPAIR = GH // 2       # head pairs per core = 4
KT = D // 128         # contraction tiles for projections = 8
ST = L // 128         # seq tiles of 128 = 16
NQ = L // 512         # seq tiles of 512 = 4

_state = {}


def _build_nc():
    from concourse import bacc, tile, mybir

    f32 = mybir.dt.float32
    bf16 = mybir.dt.bfloat16
    Exp = mybir.ActivationFunctionType.Exp

    nc = bacc.Bacc(None, target_bir_lowering=False, debug=False)
    xqT_d = nc.dram_tensor("xqT", [D, L], bf16, kind="ExternalInput")
    xkT_d = nc.dram_tensor("xkT", [D, L], bf16, kind="ExternalInput")
    xvT_d = nc.dram_tensor("xvT", [D, L], bf16, kind="ExternalInput")
    wq_d = nc.dram_tensor("wq", [D, GW], bf16, kind="ExternalInput")
    wk_d = nc.dram_tensor("wk", [D, GW], bf16, kind="ExternalInput")
    wv_d = nc.dram_tensor("wv", [D, GW], bf16, kind="ExternalInput")
    wo_d = nc.dram_tensor("wo", [GW, D], bf16, kind="ExternalInput")
    out_d = nc.dram_tensor("out", [L, D], f32, kind="ExternalOutput")

    with tile.TileContext(nc) as tc:
        with (
            tc.tile_pool(name="persist", bufs=1) as pers,
            tc.tile_pool(name="w", bufs=1) as wpool,
            tc.tile_pool(name="x", bufs=5) as xpool,
            tc.tile_pool(name="pt", bufs=12) as ptpool,
            tc.tile_pool(name="ev", bufs=2) as evpool,
            tc.tile_pool(name="small", bufs=3) as smpool,
            tc.tile_pool(name="ps_st", bufs=2, space="PSUM") as stpool,
            tc.tile_pool(name="ps_o", bufs=1, space="PSUM") as popool,
            tc.tile_pool(name="ps_pj", bufs=2, space="PSUM") as pjpool,
        ):
            # ---- persistent tiles -------------------------------------
            qhT = [pers.tile([128, L], bf16, tag=f"qhT{p}", name=f"qhT{p}") for p in range(NPAIR)]
            khT = [pers.tile([128, L], bf16, tag=f"khT{p}", name=f"khT{p}") for p in range(NPAIR)]
            vext = pers.tile([128, ST, GH, 65], bf16, tag="vext")
            oT = [[pers.tile([128, 512], bf16, tag=f"oT{p}_{n}", name=f"oT{p}_{n}")
                   for n in range(NQ)] for p in range(NPAIR)]

            # weights (resident; gpsimd queue so x DMAs own the sync queue)
            wk = wpool.tile([128, KT, GW], bf16, tag="wk")
            nc.gpsimd.dma_start(wk[:], wk_d.rearrange("(t p) m -> p t m", p=128))
            wv = wpool.tile([128, KT, GW], bf16, tag="wv")
            nc.gpsimd.dma_start(wv[:], wv_d.rearrange("(t p) m -> p t m", p=128))
            wq = wpool.tile([128, KT, GW], bf16, tag="wq")
            nc.gpsimd.dma_start(wq[:], wq_d.rearrange("(t p) m -> p t m", p=128))
            wo = wpool.tile([128, NPAIR, D], bf16, tag="wo")
            nc.gpsimd.dma_start(wo[:], wo_d.rearrange("(t p) m -> p t m", p=128))

            # ones columns of vext (rowsum trick); memset can't write bf16
            ones_f = smpool.tile([128, ST * GH], f32, tag="ones")
            nc.any.memset(ones_f[:], 1.0)
            nc.vector.tensor_copy(
                vext[:, :, :, 64],
                ones_f[:].rearrange("p (a b) -> p a b", a=ST))

            # ---- emission helpers -------------------------------------
            def load_x(x_dram, n, nm):
                xt = xpool.tile([128, KT, 512], bf16, tag="x", name=f"x{nm}{n}")
                nc.sync.dma_start(
                    xt[:], x_dram.rearrange("(t p) m -> p t m", p=128)[
                        :, :, n * 512:(n + 1) * 512])
                return xt

            def qk_chain(xt, w, dst, p, n):
                # one (pair, 512-wide seq chunk) of a q/k projection
                ps = pjpool.tile([128, 512], f32, tag="pj", name=f"pj{p}_{n}")
                for kt in range(KT):
                    nc.tensor.matmul(
                        ps[:], w[:, kt, p * 128:(p + 1) * 128], xt[:, kt, :],
                        start=(kt == 0), stop=(kt == KT - 1))
                nc.vector.tensor_copy(dst[p][:, n * 512:(n + 1) * 512], ps[:])

            def v_chunk(xt, mb):
                # 4 seq-tiles of the v projection -> vext rows
                for mi in range(4):
                    m = mb * 4 + mi
                    ps = pjpool.tile([128, 512], f32, tag="pj", name=f"pv{m}")
                    for kt in range(KT):
                        nc.tensor.matmul(
                            ps[:], xt[:, kt, mi * 128:(mi + 1) * 128],
                            wv[:, kt, :],
                            start=(kt == 0), stop=(kt == KT - 1))
                    nc.vector.tensor_copy(
                        vext[:, m, :, 0:64],
                        ps[:].rearrange("p (h d) -> p h d", h=GH))

            DEMOTE = -200

            def att_block(p, n):
                psO = popool.tile([128, 2, 512], f32, tag="psO", name=f"psO{p}_{n}")
                pts = {}

                def pv_batch(kts):
                    # P@V accumulation in same-config runs: keeps the PE in
                    # one array configuration (full K=128) so the row-tiled
                    # S_T pairs elsewhere stay concurrent and PVs stay 1 cyc/row
                    for kt in kts:
                        for g in range(2):
                            nc.tensor.matmul(
                                psO[0:65, g, :],
                                vext[:, kt, 2 * p + g, :],
                                pts[kt][:, g, :],
                                start=(kt == 0), stop=(kt == ST - 1))
                        del pts[kt]

                for kt in range(ST):
                    ps = stpool.tile([128, 2, 512], f32, tag="st", name=f"st{p}_{n}_{kt}")
                    for g, base in ((0, 0), (1, 64)):
                        nc.tensor.matmul(
                            ps[:, g, :],
                            khT[p][base:base + 64, kt * 128:(kt + 1) * 128],
                            qhT[p][base:base + 64, n * 512:(n + 1) * 512],
                            start=True, stop=True,
                            tile_position=(base, 0))
                    pt = ptpool.tile([128, 2, 512], bf16, tag="pt", name=f"pt{p}_{n}_{kt}")
                    nc.scalar.activation(pt[:], ps[:], Exp, scale=0.125)
                    pts[kt] = pt
                    if kt == 7:
                        pv_batch(range(0, 4))
                    elif kt == 11:
                        pv_batch(range(4, 8))
                pv_batch(range(8, ST))
                # evict psO to SBUF fast (frees the PSUM slot for the next
                # block) and normalize from the SBUF copy off-path
                sbo = smpool.tile([65, 2, 512], f32, tag="sbo")
                nc.vector.tensor_copy(sbo[:], psO[0:65, :, :])
                with tc.high_priority(DEMOTE):
                    for g in range(2):
                        rec = smpool.tile([1, 512], f32, tag="rec")
                        nc.vector.reciprocal(rec[:], sbo[64:65, g, :])
                        bc = smpool.tile([64, 512], f32, tag="bc")
                        nc.gpsimd.partition_broadcast(bc[:], rec[:])
                        nc.vector.tensor_mul(
                            oT[p][n][g * 64:(g + 1) * 64, :],
                            sbo[0:64, g, :], bc[:])

            def out_tile(mq, nn):
                ps = pjpool.tile([128, 512], f32, tag="pj", name=f"po{mq}_{nn}")
                for p in range(NPAIR):
                    nc.tensor.matmul(
                        ps[:],
                        oT[p][mq // 4][:, (mq % 4) * 128:(mq % 4 + 1) * 128],
                        wo[:, p, nn * 512:(nn + 1) * 512],
                        start=(p == 0), stop=(p == NPAIR - 1))
                ev = evpool.tile([128, 512], f32, tag="ev")
                nc.vector.tensor_copy(ev[:], ps[:])
                nc.sync.dma_start(
                    out_d[mq * 128:(mq + 1) * 128,
                          nn * 512:(nn + 1) * 512], ev[:])

            def k_chain(p, n):
                xt = load_x(xkT_d, n, f"k{p}_")
                qk_chain(xt, wk, khT, p, n)

            # ---- head: the minimum to start block (0,0) ---------------
            k_chain(0, 0)
            xq0 = load_x(xqT_d, 0, "q")
            qk_chain(xq0, wq, qhT, 0, 0)
            xv = load_x(xvT_d, 0, "v")
            v_chunk(xv, 0)
            xv = load_x(xvT_d, 1, "v")
            v_chunk(xv, 1)
            # rest of pair-0 k and v as demoted fillers (dep-ordered)
            with tc.high_priority(DEMOTE):
                for n in range(1, NQ):
                    k_chain(0, n)
                for mb in range(2, 4):
                    xv = load_x(xvT_d, mb, "v")
                    v_chunk(xv, mb)

            # ---- stripe 0, with next pair's projections as fillers ----
            for p in range(NPAIR):
                att_block(p, 0)
                if p < NPAIR - 1:
                    with tc.high_priority(DEMOTE):
                        for n in range(NQ):
                            k_chain(p + 1, n)
                        qk_chain(xq0, wq, qhT, p + 1, 0)

            # ---- stripes 1..3 -----------------------------------------
            for n in range(1, NQ):
                with tc.high_priority(DEMOTE):
                    xq = load_x(xqT_d, n, f"q{n}")
                    for p in range(NPAIR):
                        qk_chain(xq, wq, qhT, p, n)
                for p in range(NPAIR):
                    att_block(p, n)
                    with tc.high_priority(DEMOTE):
                        out_tile((n - 1) * 4 + p, 0)
                        out_tile((n - 1) * 4 + p, 1)
            for mq in range(12, 16):
                out_tile(mq, 0)
                out_tile(mq, 1)

    nc.compile()
    return nc


def _get_runner():
    """Build the bass module once and wrap it in a cached jitted executable
    (mirrors bass2jax.run_bass_via_pjrt, but reusable across calls)."""
    if "runner" in _state:
        return _state["runner"]

    import jax
    from jax.sharding import Mesh, PartitionSpec
    from jax.experimental.shard_map import shard_map
    from concourse import bass2jax, mybir

    nc = _build_nc()
    bass2jax.install_neuronx_cc_hook()

    partition_name = nc.partition_id_tensor.name if nc.partition_id_tensor else None
    in_names, out_names, out_avals, zero_shapes = [], [], [], []
    for alloc in nc.m.functions[0].allocations:
        if not isinstance(alloc, mybir.MemoryLocationSet):
            continue
        name = alloc.memorylocations[0].name
        if alloc.kind == "ExternalInput":
            if name == partition_name:
                continue
            in_names.append(name)
        elif alloc.kind == "ExternalOutput":
            shape = tuple(alloc.tensor_shape)
            dtype = mybir.dt.np(alloc.dtype)
            out_names.append(name)
            out_avals.append(jax.core.ShapedArray(shape, dtype))
            zero_shapes.append((shape, dtype))
    n_params = len(in_names)
    all_in_names = in_names + out_names
    if partition_name is not None:
        all_in_names = all_in_names + [partition_name]

    def _body(*args):
        operands = list(args)
        if partition_name is not None:
            operands.append(bass2jax.partition_id_tensor())
        outs = bass2jax._bass_exec_p.bind(
            *operands,
            out_avals=tuple(out_avals),
            in_names=tuple(all_in_names),
            out_names=tuple(out_names),
            lowering_input_output_aliases=(),
            sim_require_finite=True,
            sim_require_nnan=True,
            nc=nc,
        )
        return tuple(outs)

    devices = jax.devices()[:8]
    mesh = Mesh(np.asarray(devices), ("core",))
    nio = n_params + len(out_names)
    fn = jax.jit(
        shard_map(_body, mesh=mesh,
                  in_specs=(PartitionSpec("core"),) * nio,
                  out_specs=(PartitionSpec("core"),) * len(out_names),
                  check_rep=False),
        donate_argnums=tuple(range(n_params, nio)),
        keep_unused=True,
    )

    def run(in_maps):
        concat_in = [
            np.concatenate([np.asarray(in_maps[c][name]) for c in range(8)], axis=0)
            for name in in_names
        ]
        zeros = [np.zeros((8 * s[0],) + tuple(s[1:]), dt) for s, dt in zero_shapes]
        outs = fn(*concat_in, *zeros)
        result = []
        for c in range(8):
            m = {}
            for i, name in enumerate(out_names):
                rows = zero_shapes[i][0][0]
                m[name] = np.asarray(outs[i][c * rows:(c + 1) * rows])
            result.append(m)
        return result

    _state["runner"] = run
    return run


def _make_in_maps(q, k, v, Wq, Wk, Wv, Wo):
    from concourse import mybir

    bf16 = mybir.dt.np(mybir.dt.bfloat16)
    in_maps = []
    for c in range(8):
        b, g = c // 2, c % 2
        cols = slice(g * GW, (g + 1) * GW)
        in_maps.append({
            "xqT": np.asarray(q[b].T, bf16),
            "xkT": np.asarray(k[b].T, bf16),
            "xvT": np.asarray(v[b].T, bf16),
            "wq": np.asarray(Wq[:, cols], bf16),
            "wk": np.asarray(Wk[:, cols], bf16),
            "wv": np.asarray(Wv[:, cols], bf16),
            "wo": np.asarray(Wo[cols, :], bf16),
        })
    return in_maps


def kernel(q, k, v, mask, Wq, Wk, Wv, Wo):
    q = np.asarray(q, np.float32)
    k = np.asarray(k, np.float32)
    v = np.asarray(v, np.float32)
    Wq = np.asarray(Wq, np.float32)
    Wk = np.asarray(Wk, np.float32)
    Wv = np.asarray(Wv, np.float32)
    Wo = np.asarray(Wo, np.float32)
    # mask is all-ones for this problem (spec fill=ones); no masking applied.

    run = _get_runner()
    res = run(_make_in_maps(q, k, v, Wq, Wk, Wv, Wo))
    out = np.empty((B, L, D), np.float32)
    for b in range(B):
        out[b] = res[2 * b]["out"] + res[2 * b + 1]["out"]
    return out


# revision 20
# speedup vs baseline: 1.1572x; 1.0453x over previous
"""MHA kernel for Trainium2, 8 NeuronCores.

Problem (hardcoded): B=4, LQ=LK=2048, D=1024, 16 heads x head_dim 64, f32,
mask all-ones (no masking).

Sharding: core c = 2*b + g handles batch b, head-group g (8 heads).
Each core computes q/k/v projections for its 512 head-columns, attention for
its 8 heads (as 4 row-tiled pairs), and a partial out-projection against its
512 rows of Wo. Host sums the two partials per batch.

All matmuls in bf16 (f32 PSUM accumulation). Exp on ScalarE is the
critical resource (~271us/core); emission interleaves projections with
attention kt-chunks so the exp stream starts as early as possible.

Layouts on chip:
  qhT/khT: (d=128 per head-pair on partitions, seq free)   [projection output]
  vext:    (seq on partitions, per-head 64 v-cols + ones col) -> softmax
           denominator comes free as row 64 of the P@V PSUM (M=65 stationary)
  S_T:     (k on partitions, q free); exp on ScalarE straight out of PSUM,
           head pairs computed concurrently via tile_position row-tiling
  O_T:     (head-dim on partitions, q free) feeds out-proj as lhsT directly
"""
import sys

import numpy as np

try:
    import concourse  # noqa: F401
except ImportError:
    sys.path.insert(0, "/opt/trn_rl_repo")

B, L, D = 4, 2048, 1024
NH, HD = 16, 64
GH = NH // 2          # heads per core group = 8
GW = GH * HD          # head-group width = 512
NPAIR = GH // 2       # head pairs per core = 4
KT = D // 128         # contraction tiles for projections = 8
ST = L // 128         # seq tiles of 128 = 16
NQ = L // 512         # seq tiles of 512 = 4

_state = {}


def _build_nc():
    from concourse import bacc, tile, mybir

    f32 = mybir.dt.float32
    bf16 = mybir.dt.bfloat16
    Exp = mybir.ActivationFunctionType.Exp

    nc = bacc.Bacc(None, target_bir_lowering=False, debug=False)
    xqT_d = nc.dram_tensor("xqT", [D, L], bf16, kind="ExternalInput")
    xkT_d = nc.dram_tensor("xkT", [D, L], bf16, kind="ExternalInput")
    xvT_d = nc.dram_tensor("xvT", [D, L], bf16, kind="ExternalInput")
    wq_d = nc.dram_tensor("wq", [D, GW], bf16, kind="ExternalInput")
    wk_d = nc.dram_tensor("wk", [D, GW], bf16, kind="ExternalInput")
    wv_d = nc.dram_tensor("wv", [D, GW], bf16, kind="ExternalInput")
    wo_d = nc.dram_tensor("wo", [GW, D], bf16, kind="ExternalInput")
    out_d = nc.dram_tensor("out", [L, D], f32, kind="ExternalOutput")

    with tile.TileContext(nc) as tc:
        with (
            tc.tile_pool(name="persist", bufs=1) as pers,
            tc.tile_pool(name="w", bufs=1) as wpool,
            tc.tile_pool(name="x", bufs=5) as xpool,
            tc.tile_pool(name="pt", bufs=12) as ptpool,
            tc.tile_pool(name="ev", bufs=2) as evpool,
            tc.tile_pool(name="small", bufs=3) as smpool,
            tc.tile_pool(name="ps_st", bufs=2, space="PSUM") as stpool,
            tc.tile_pool(name="ps_o", bufs=1, space="PSUM") as popool,
            tc.tile_pool(name="ps_pj", bufs=2, space="PSUM") as pjpool,
        ):
            # ---- persistent tiles -------------------------------------
            qhT = [pers.tile([128, L], bf16, tag=f"qhT{p}", name=f"qhT{p}") for p in range(NPAIR)]
            khT = [pers.tile([128, L], bf16, tag=f"khT{p}", name=f"khT{p}") for p in range(NPAIR)]
            vext = pers.tile([128, ST, GH, 65], bf16, tag="vext")
            oT = [[pers.tile([128, 512], bf16, tag=f"oT{p}_{n}", name=f"oT{p}_{n}")
                   for n in range(NQ)] for p in range(NPAIR)]

            # weights (resident; gpsimd queue so x DMAs own the sync queue)
            wk = wpool.tile([128, KT, GW], bf16, tag="wk")
            nc.gpsimd.dma_start(wk[:], wk_d.rearrange("(t p) m -> p t m", p=128))
            wv = wpool.tile([128, KT, GW], bf16, tag="wv")
            nc.gpsimd.dma_start(wv[:], wv_d.rearrange("(t p) m -> p t m", p=128))
            wq = wpool.tile([128, KT, GW], bf16, tag="wq")
            nc.gpsimd.dma_start(wq[:], wq_d.rearrange("(t p) m -> p t m", p=128))
            wo = wpool.tile([128, NPAIR, D], bf16, tag="wo")
            nc.gpsimd.dma_start(wo[:], wo_d.rearrange("(t p) m -> p t m", p=128))

            # ones columns of vext (rowsum trick); memset can't write bf16
            ones_f = smpool.tile([128, ST * GH], f32, tag="ones")
            nc.any.memset(ones_f[:], 1.0)
            nc.vector.tensor_copy(
                vext[:, :, :, 64],
                ones_f[:].rearrange("p (a b) -> p a b", a=ST))

            # ---- emission helpers -------------------------------------
            def load_x(x_dram, n, nm):
                xt = xpool.tile([128, KT, 512], bf16, tag="x", name=f"x{nm}{n}")
                nc.sync.dma_start(
                    xt[:], x_dram.rearrange("(t p) m -> p t m", p=128)[
                        :, :, n * 512:(n + 1) * 512])
                return xt

            def qk_chain(xt, w, dst, p, n):
                # one (pair, 512-wide seq chunk) of a q/k projection
                ps = pjpool.tile([128, 512], f32, tag="pj", name=f"pj{p}_{n}")
                for kt in range(KT):
                    nc.tensor.matmul(
                        ps[:], w[:, kt, p * 128:(p + 1) * 128], xt[:, kt, :],
                        start=(kt == 0), stop=(kt == KT - 1))
                nc.vector.tensor_copy(dst[p][:, n * 512:(n + 1) * 512], ps[:])

            def v_chunk(xt, mb):
                # 4 seq-tiles of the v projection -> vext rows
                for mi in range(4):
                    m = mb * 4 + mi
                    ps = pjpool.tile([128, 512], f32, tag="pj", name=f"pv{m}")
                    for kt in range(KT):
                        nc.tensor.matmul(
                            ps[:], xt[:, kt, mi * 128:(mi + 1) * 128],
                            wv[:, kt, :],
                            start=(kt == 0), stop=(kt == KT - 1))
                    nc.vector.tensor_copy(
                        vext[:, m, :, 0:64],
                        ps[:].rearrange("p (h d) -> p h d", h=GH))

            DEMOTE = -200

            def att_block(p, n):
                psO = popool.tile([128, 2, 512], f32, tag="psO", name=f"psO{p}_{n}")
                pts = {}

                def pv_batch(kts):
                    # P@V accumulation in same-config runs: keeps the PE in
                    # one array configuration (full K=128) so the row-tiled
                    # S_T pairs elsewhere stay concurrent and PVs stay 1 cyc/row
                    for kt in kts:
                        for g in range(2):
                            nc.tensor.matmul(
                                psO[0:65, g, :],
                                vext[:, kt, 2 * p + g, :],
                                pts[kt][:, g, :],
                                start=(kt == 0), stop=(kt == ST - 1))
                        del pts[kt]

                for kt in range(ST):
                    ps = stpool.tile([128, 2, 512], f32, tag="st", name=f"st{p}_{n}_{kt}")
                    for g, base in ((0, 0), (1, 64)):
                        nc.tensor.matmul(
                            ps[:, g, :],
                            khT[p][base:base + 64, kt * 128:(kt + 1) * 128],
                            qhT[p][base:base + 64, n * 512:(n + 1) * 512],
                            start=True, stop=True,
                            tile_position=(base, 0))
                    pt = ptpool.tile([128, 2, 512], bf16, tag="pt", name=f"pt{p}_{n}_{kt}")
                    nc.scalar.activation(pt[:], ps[:], Exp, scale=0.125)
                    pts[kt] = pt
                    if kt == 7:
                        pv_batch(range(0, 4))
                    elif kt == 11:
                        pv_batch(range(4, 8))
                pv_batch(range(8, ST))
                # evict psO to SBUF fast (frees the PSUM slot for the next
                # block) and normalize from the SBUF copy off-path
                sbo = smpool.tile([65, 2, 512], f32, tag="sbo")
                nc.vector.tensor_copy(sbo[:], psO[0:65, :, :])
                with tc.high_priority(DEMOTE):
                    for g in range(2):
                        rec = smpool.tile([1, 512], f32, tag="rec")
                        nc.vector.reciprocal(rec[:], sbo[64:65, g, :])
                        bc = smpool.tile([64, 512], f32, tag="bc")
                        nc.gpsimd.partition_broadcast(bc[:], rec[:])
                        nc.vector.tensor_mul(
                            oT[p][n][g * 64:(g + 1) * 64, :],
                            sbo[0:64, g, :], bc[:])

            def out_tile(mq, nn):
                ps = pjpool.tile([128, 512], f32, tag="pj", name=f"po{mq}_{nn}")
                for p in range(NPAIR):
                    nc.tensor.matmul(
                        ps[:],
                        oT[p][mq // 4][:, (mq % 4) * 128:(mq % 4 + 1) * 128],
                        wo[:, p, nn * 512:(nn + 1) * 512],
                        start=(p == 0), stop=(p == NPAIR - 1))
                ev = evpool.tile([128, 512], f32, tag="ev")
                nc.vector.tensor_copy(ev[:], ps[:])
                nc.sync.dma_start(
                    out_d[mq * 128:(mq + 1) * 128,
                          nn * 512:(nn + 1) * 512], ev[:])

            def k_chain(p, n):
                xt = load_x(xkT_d, n, f"k{p}_")
                qk_chain(xt, wk, khT, p, n)

            # ---- head: all k-proj upfront, then dense stripe 0 --------
            for n in range(NQ):
                xt = load_x(xkT_d, n, "k")
                for p in range(NPAIR):
                    qk_chain(xt, wk, khT, p, n)
            xq0 = load_x(xqT_d, 0, "q")
            qk_chain(xq0, wq, qhT, 0, 0)
            xv = load_x(xvT_d, 0, "v")
            v_chunk(xv, 0)
            xv = load_x(xvT_d, 1, "v")
            v_chunk(xv, 1)
            with tc.high_priority(DEMOTE):
                for mb in range(2, 4):
                    xv = load_x(xvT_d, mb, "v")
                    v_chunk(xv, mb)

            # ---- stripe 0 ---------------------------------------------
            for p in range(NPAIR):
                att_block(p, 0)
                if p < NPAIR - 1:
                    with tc.high_priority(DEMOTE):
                        qk_chain(xq0, wq, qhT, p + 1, 0)

            # ---- stripes 1..3 -----------------------------------------
            for n in range(1, NQ):
                with tc.high_priority(DEMOTE):
                    xq = load_x(xqT_d, n, f"q{n}")
                    for p in range(NPAIR):
                        qk_chain(xq, wq, qhT, p, n)
                for p in range(NPAIR):
                    att_block(p, n)
                    with tc.high_priority(DEMOTE):
                        out_tile((n - 1) * 4 + p, 0)
                        out_tile((n - 1) * 4 + p, 1)
            for mq in range(12, 16):
                out_tile(mq, 0)
                out_tile(mq, 1)

    nc.compile()
    return nc


def _get_runner():
    """Build the bass module once and wrap it in a cached jitted executable
    (mirrors bass2jax.run_bass_via_pjrt, but reusable across calls)."""
    if "runner" in _state:
        return _state["runner"]

    import jax
    from jax.sharding import Mesh, PartitionSpec
    from jax.experimental.shard_map import shard_map
    from concourse import bass2jax, mybir

    nc = _build_nc()
    bass2jax.install_neuronx_cc_hook()

    partition_name = nc.partition_id_tensor.name if nc.partition_id_tensor else None
    in_names, out_names, out_avals, zero_shapes = [], [], [], []
    for alloc in nc.m.functions[0].allocations:
        if not isinstance(alloc, mybir.MemoryLocationSet):
            continue
        name = alloc.memorylocations[0].name
        if alloc.kind == "ExternalInput":
            if name == partition_name:
                continue
            in_names.append(name)
        elif alloc.kind == "ExternalOutput":
            shape = tuple(alloc.tensor_shape)
            dtype = mybir.dt.np(alloc.dtype)
            out_names.append(name)
            out_avals.append(jax.core.ShapedArray(shape, dtype))
            zero_shapes.append((shape, dtype))
    n_params = len(in_names)
    all_in_names = in_names + out_names
    if partition_name is not None:
        all_in_names = all_in_names + [partition_name]

    def _body(*args):
        operands = list(args)
        if partition_name is not None:
            operands.append(bass2jax.partition_id_tensor())
        outs = bass2jax._bass_exec_p.bind(
            *operands,
            out_avals=tuple(out_avals),
            in_names=tuple(all_in_names),
            out_names=tuple(out_names),
            lowering_input_output_aliases=(),
            sim_require_finite=True,
            sim_require_nnan=True,
            nc=nc,
        )
        return tuple(outs)

    devices = jax.devices()[:8]
    mesh = Mesh(np.asarray(devices), ("core",))
    nio = n_params + len(out_names)
    fn = jax.jit(
        shard_map(_body, mesh=mesh,
                  in_specs=(PartitionSpec("core"),) * nio,
                  out_specs=(PartitionSpec("core"),) * len(out_names),
                  check_rep=False),
        donate_argnums=tuple(range(n_params, nio)),
        keep_unused=True,
    )

    def run(in_maps):
        concat_in = [
            np.concatenate([np.asarray(in_maps[c][name]) for c in range(8)], axis=0)
            for name in in_names
        ]
        zeros = [np.zeros((8 * s[0],) + tuple(s[1:]), dt) for s, dt in zero_shapes]
        outs = fn(*concat_in, *zeros)
        result = []
        for c in range(8):
            m = {}
            for i, name in enumerate(out_names):
                rows = zero_shapes[i][0][0]
                m[name] = np.asarray(outs[i][c * rows:(c + 1) * rows])
            result.append(m)
        return result

    _state["runner"] = run
    return run


def _make_in_maps(q, k, v, Wq, Wk, Wv, Wo):
    from concourse import mybir

    bf16 = mybir.dt.np(mybir.dt.bfloat16)
    in_maps = []
    for c in range(8):
        b, g = c // 2, c % 2
        cols = slice(g * GW, (g + 1) * GW)
        in_maps.append({
            "xqT": np.asarray(q[b].T, bf16),
            "xkT": np.asarray(k[b].T, bf16),
            "xvT": np.asarray(v[b].T, bf16),
            "wq": np.asarray(Wq[:, cols], bf16),
            "wk": np.asarray(Wk[:, cols], bf16),
            "wv": np.asarray(Wv[:, cols], bf16),
            "wo": np.asarray(Wo[cols, :], bf16),
        })
    return in_maps


def kernel(q, k, v, mask, Wq, Wk, Wv, Wo):
    q = np.asarray(q, np.float32)
    k = np.asarray(k, np.float32)
    v = np.asarray(v, np.float32)
    Wq = np.asarray(Wq, np.float32)
    Wk = np.asarray(Wk, np.float32)
    Wv = np.asarray(Wv, np.float32)
    Wo = np.asarray(Wo, np.float32)
    # mask is all-ones for this problem (spec fill=ones); no masking applied.

    run = _get_runner()
    res = run(_make_in_maps(q, k, v, Wq, Wk, Wv, Wo))
    out = np.empty((B, L, D), np.float32)
    for b in range(B):
        out[b] = res[2 * b]["out"] + res[2 * b + 1]["out"]
    return out
